# revision 52
# baseline (speedup 1.0000x reference)
"""Trainium2 Bass kernel for BatchWiseTripletDistanceLoss.

Math: loss = mean_t relu(cos_d(s[a_t], s[p_t]) - cos_d(s[a_t], s[n_t]) + margin)
with cos_d(x, y) = 1 - <x,y>/max(|x||y|, eps).  The "1-" cancels in the
difference; with d[q, j] = <s_q, s_j>/|s_j| (note: NOT normalized by |s_q|)
each triplet term is (1/|s_q|) * relu(d[a,n] - d[a,p] + margin*|s_q|), so the
anchor norm folds into a per-partition margin (mcol) and a per-partition
output scale (rqhalf), keeping it off the critical path.

Device algorithm (per core; grid = 128 anchor rows x 256 negative columns):
  - st is an f8e4 copy of samples in a per-core column permutation `perm`
    placing the core's own 128 rows at 192:320 and the core's negative half
    at 0:256; layout [128 d-partitions, 1024] = two 512-col halves for
    d=0:128 / d=128:256.  f8 halves the compute-gating input transfer; its
    rounding is an input perturbation worth ~6e-4 on the loss.  The f16
    squares of f8 values are exact, so norms match the vectors used in sim.
  - squares are f8, column-split at 320 (own block 192:320 whole in piece
    A); each square op writes both k-blocks of its column range through
    strided 3-d views, so each n2 piece is ONE DoubleRow matmul and the
    norm pipeline (square -> n2 -> rsqrt -> C16) starts after only the
    first square op.  sim = (own rows)^T (all rows) in ONE DoubleRow f8
    matmul; rq2p = sqA[:,own-block]^T ones transposes the own-row norms^2
    to a column.
  - rjb = rsqrt(n2) on Scalar (the only ACT function; table preloaded via
    a dummy activation), then C16 = sim * rjb on DVE, in column pieces so
    C16 piece 0 overlaps rjb piece 1.  Side path (rqcol, mcol = m*|s_q|,
    rqhalf = 0.5/|s_q|) is emitted after the scan so its DVE stream
    position stays past the scatter's semaphore threshold.
  - ONE gpsimd local_scatter fills a [128, 512] slot grid from C16[:,0:514]
    (p- and n-source columns are disjoint per row since labels differ):
    dst[:,0:256] ("buk") gets d[a,n] per triplet slot (triplets of a row
    sorted by positive id -> equal-positive runs are contiguous slots);
    dst[:,256:512] ("vgrid") gets d[a,p] at each run start, plus a sentinel
    +BIG just past the last slot (source col 512).
  - DVE prefix scan  state = keep * state - vgrid  (keep = 0 at run starts)
    forward-fills -d[a,p] across each run; tail slots get -BIG.
  - relu without an activation: sum relu(y) = (sum y + sum |y|)/2 with
    y = buk + mcol + bias.  scalar_tensor_tensor computes y and sum y per
    partition; tensor_reduce(abs) gives sum |y|; a matmul with stationary
    rqhalf reduces both columns across partitions to [1,2], copied to SBUF
    and DMAd out (the out DMA rides the scalar queue - the sync queue's
    closing drain otherwise serializes behind it).  BIG = 4 keeps tail
    terms small so the f32 sums stay precise.

Host does layout/indexing only (permutations, bucketing, run starts);
all floating-point math runs on device.

Sharding: 8 cores = (anchor row mod 4) x (negative column half).
"""
import sys

sys.path.insert(0, "/opt/trn_rl_repo")

from contextlib import ExitStack

import ml_dtypes
import numpy as np

import concourse.bacc as bacc
import concourse.bass as bass
import concourse.tile as tile
from concourse import mybir
from concourse.bass_utils import run_bass_kernel_spmd

DT = mybir.dt
OP = mybir.AluOpType
ACTF = mybir.ActivationFunctionType

N = 512
D = 256
MARGIN = 0.15
NCORES = 8
NROW = 128  # anchor rows per core
LCOL = 256  # negative columns per core
BIG = 4.0  # tail-slot poison; > max margin*|s_q|, small so f32 sums stay exact
WPB = 514 + 256  # idx2 | keep


def _build_program():
    nc = bacc.Bacc(
        "TRN2", target_bir_lowering=False, debug=False, num_devices=NCORES
    )
    f32, i16, f16 = DT.float32, DT.int16, DT.float16

    f8 = DT.float8e4
    d_pa = nc.dram_tensor("packa", [128, 1024], f8, kind="ExternalInput").ap()
    d_pb = nc.dram_tensor("packb", [NROW, WPB], i16, kind="ExternalInput").ap()
    d_out = nc.dram_tensor("out", [1, 2], f32, kind="ExternalOutput").ap()

    with tile.TileContext(nc) as tc, ExitStack() as ctx:
        cpool = ctx.enter_context(tc.tile_pool(name="const", bufs=1))
        ppool = ctx.enter_context(tc.tile_pool(name="psum", bufs=1, space="PSUM"))
        pbig = ctx.enter_context(tc.tile_pool(name="psumbig", bufs=1, space="PSUM"))

        # ---- inputs: f8 samples (halves the compute-gating transfer), ----
        # pb BEHIND st on the same HW queues (queue FIFO keeps st first).
        # The gpsimd queue carries NO DMA: a software-DGE DMA there forces
        # a queue drain on the gpsimd engine right before the scatter.
        st = cpool.tile([128, 1024], f8)
        nc.sync.dma_start(st[0:64, :], d_pa[0:64, :])
        nc.scalar.dma_start(st[64:128, :], d_pa[64:128, :])
        pb = cpool.tile([NROW, WPB], i16)
        nc.sync.dma_start(pb[0:64, :], d_pb[0:64, :])
        nc.scalar.dma_start(pb[64:128, :], d_pb[64:128, :])
        idx2 = pb[:, 0:514]
        keepg = pb[:, 514:770].bitcast(f16)

        # ---- constants (DVE) + rsqrt ACT table preload (Scalar) ----------
        onesmat8 = cpool.tile([128, 256], f8)
        nc.vector.memset(onesmat8[:], 1.0)
        C16 = cpool.tile([128, 514], f16)
        nc.vector.memset(C16[:, 512:514], BIG)
        dumin = cpool.tile([1, 1], f32)
        nc.vector.memset(dumin[:], 4.0)
        dum1 = cpool.tile([1, 1], f32)
        nc.scalar.activation(dum1[:], dumin[:], ACTF.Abs_reciprocal_sqrt)
        # gpsimd warmup: the first ucode op on the Pool queue pays a cold
        # launch (~300ns observed on the scatter); a dep-free memset long
        # before the scatter absorbs it (back-to-back gpsimd ops launch in
        # ~15ns per the baseline's two-scatter trace)
        gwarm = cpool.tile([128, 2], f16)
        nc.gpsimd.memset(gwarm[:], 0.0)

        # ---- squares -> norms, pipelined by column pieces ----------------
        # f8 squares of the f8 samples: n2 noise acts multiplicatively on C
        # (~1e-4 class on the loss).  Column split at 320 keeps the own
        # block (192:320) whole in sqA (its DoubleRow transpose needs a
        # full 256-wide stationary).  Each square op writes BOTH k-blocks
        # of its column range via strided 3-d views, so each n2 piece is a
        # single DoubleRow matmul and rjb piece 0 starts after only the
        # first square op.
        DR = mybir.MatmulPerfMode.DoubleRow
        st3 = st[:].rearrange("p (two f) -> p two f", two=2)
        ones3 = onesmat8[:].rearrange("p (two f) -> p two f", two=2)
        sqA = cpool.tile([128, 640], f8)
        sqB = cpool.tile([128, 384], f8)
        sqA3 = sqA[:].rearrange("p (two f) -> p two f", two=2)
        sqB3 = sqB[:].rearrange("p (two f) -> p two f", two=2)
        nc.vector.tensor_tensor(sqA3, st3[:, :, 0:320], st3[:, :, 0:320], OP.mult)
        nc.vector.tensor_tensor(
            sqB3, st3[:, :, 320:512], st3[:, :, 320:512], OP.mult
        )
        n2bA = pbig.tile([128, 320], f32, tag="n2bA")
        n2bB = pbig.tile([128, 192], f32, tag="n2bB")
        with tc.high_priority():
            nc.tensor.matmul(n2bA[:], ones3, sqA3, start=True, stop=True, perf_mode=DR)
            nc.tensor.matmul(n2bB[:], ones3, sqB3, start=True, stop=True, perf_mode=DR)
        # sim in ONE DoubleRow f8 matmul: both 128-row contraction halves
        # ride in a single instruction at 2 MACs/cycle
        simp = pbig.tile([128, 512], f32, tag="simp")
        nc.tensor.matmul(
            simp[:], st3[:, :, 192:320], st3, start=True, stop=True, perf_mode=DR
        )
        rq2p = ppool.tile([128, 2], f32, tag="rq2p")
        nc.tensor.matmul(
            rq2p[:], sqA3[:, :, 192:320], ones3[:, :, 0:2],
            start=True, stop=True, perf_mode=DR,
        )

        # rjb/C16 in column pieces so C16 piece 0 overlaps rjb piece 1
        rjb16 = cpool.tile([128, 512], f16)
        rqcol = cpool.tile([128, 1], f32)
        with tc.high_priority():
            for c, n2h in ((slice(0, 320), n2bA), (slice(320, 512), n2bB)):
                nc.scalar.activation(rjb16[:, c], n2h[:], ACTF.Abs_reciprocal_sqrt)
                nc.vector.tensor_tensor(C16[:, c], simp[:, c], rjb16[:, c], OP.mult)
        nc.scalar.activation(rqcol[:], rq2p[:, 0:1], ACTF.Abs_reciprocal_sqrt)

        # ---- one scatter fills buk (0:256) and vgrid (256:512) -----------
        dst = cpool.tile([NROW, 512], f16)
        nc.gpsimd.local_scatter(
            dst[:], C16[:], idx2, channels=128, num_elems=512, num_idxs=514
        )

        # ---- scan fills -d[a,p]; relu via (y + |y|)/2 --------------------
        # f16 biasg/y keep the DVE in 2x mode; |values| <= BIG so rounding
        # is ~1e-3 relative and unbiased
        biasg = cpool.tile([NROW, LCOL], f16)
        nc.vector.tensor_tensor_scan(
            biasg[:], keepg, dst[:, 256:512], 0.0, OP.mult, OP.subtract
        )
        # anchor-norm side path, emitted AFTER the scan so its DVE stream
        # position is past the scatter's semaphore threshold (the scatter
        # waits on a DVE completion count; any small op scheduled before
        # C16 piece 1 pushes the scatter start by that op's latency).
        # Dependencies still force mcol before the y pass below.
        rqrecip = cpool.tile([128, 1], f32)
        nc.vector.reciprocal(rqrecip[:], rqcol[:])
        mcol = cpool.tile([128, 1], f32)
        nc.vector.tensor_scalar(mcol[:], rqrecip[:], MARGIN, 0.0, OP.mult, OP.add)
        rqhalf = cpool.tile([128, 1], f32)
        nc.vector.tensor_scalar(rqhalf[:], rqcol[:], 0.5, 0.0, OP.mult, OP.add)
        y = cpool.tile([NROW, LCOL], f16)
        acc2 = cpool.tile([NROW, 2], f32)
        nc.vector.scalar_tensor_tensor(
            y[:], dst[:, 0:256], mcol[:], biasg[:], OP.add, OP.add,
            accum_out=acc2[:, 0:1],
        )
        nc.vector.tensor_reduce(
            acc2[:, 1:2], y[:], mybir.AxisListType.X, OP.add,
            apply_absolute_value=True,
        )
        totp = ppool.tile([1, 2], f32, tag="totp")
        nc.tensor.matmul(totp[:], rqhalf[:], acc2[:], start=True, stop=True)
        tot = cpool.tile([1, 2], f32)
        nc.vector.tensor_scalar(tot[:], totp[:], 1.0, 0.0, OP.mult, OP.add)
        nc.scalar.dma_start(d_out, tot[:], single_packet=True)

    nc.compile()
    return nc


_PROGRAM = None


def _get_program():
    global _PROGRAM
    if _PROGRAM is None:
        _PROGRAM = _build_program()
    return _PROGRAM


def _shard_inputs(samples, a, p, n):
    """Per-core layout: permute samples, bucket triplets (sorted by positive
    id so equal-positive slots are contiguous runs), build scatter indices."""
    in_maps = []
    allr = np.arange(N, dtype=np.int64)
    for core in range(NCORES):
        R, H = core >> 1, core & 1
        inH = (allr >> 8) == H
        ownm = (allr & 3) == R
        own_H = allr[ownm & inH]          # 64
        own_O = allr[ownm & ~inH]         # 64
        non_own_H = allr[~ownm & inH]     # 192
        non_own_O = allr[~ownm & ~inH]    # 192
        perm = np.concatenate([non_own_H, own_H, own_O, non_own_O])
        colpos = np.empty(N, dtype=np.int64)
        colpos[perm] = np.arange(N)
        rows_core = np.concatenate([own_H, own_O])  # partition q -> global row
        qof = np.full(N, -1, dtype=np.int64)
        qof[rows_core] = np.arange(NROW)

        sel = ((a & 3) == R) & ((n >> 8) == H)
        asel, psel, nsel = a[sel], p[sel], n[sel]
        q = qof[asel]
        order = np.lexsort((psel, q))
        qs, ps, ns = q[order], psel[order], nsel[order]
        counts = np.bincount(qs, minlength=NROW)
        starts = np.zeros(NROW, dtype=np.int64)
        starts[1:] = np.cumsum(counts)[:-1]
        slot = np.arange(len(qs)) - starts[qs]  # slot within row (sorted by p)

        idx2 = np.full((NROW, 514), -1, dtype=np.int16)
        idx2[qs, colpos[ns]] = slot.astype(np.int16)
        if len(qs):
            newrun = np.ones(len(qs), dtype=bool)
            newrun[1:] = (qs[1:] != qs[:-1]) | (ps[1:] != ps[:-1])
        else:
            newrun = np.zeros(0, dtype=bool)
        idx2[qs[newrun], colpos[ps[newrun]]] = (256 + slot[newrun]).astype(np.int16)
        has_room = counts < 256
        idx2[has_room, 512] = (256 + counts[has_room]).astype(np.int16)

        keep = np.ones((NROW, 256), dtype=np.float16)
        keep[qs[newrun], slot[newrun]] = 0.0
        keep[has_room, np.minimum(counts, 255)[has_room]] = 0.0

        A8 = samples[perm].astype(ml_dtypes.float8_e4m3)  # [512, 256]
        AT = np.ascontiguousarray(A8.T)  # [256, 512] = (d, col)
        packa = np.concatenate([AT[0:128, :], AT[128:256, :]], axis=1)
        in_maps.append(
            {
                "packa": np.ascontiguousarray(packa),
                "packb": np.concatenate([idx2, keep.view(np.int16)], axis=1),
            }
        )
    return in_maps


def kernel(samples, targets, anchor_idx, pos_idx, neg_idx, _want_trace=False):
    samples = np.asarray(samples, dtype=np.float32)
    targets = np.asarray(targets)
    a = np.asarray(anchor_idx).astype(np.int64)
    p = np.asarray(pos_idx).astype(np.int64)
    n = np.asarray(neg_idx).astype(np.int64)
    T = a.shape[0]
    assert samples.shape == (N, D)

    ok = (
        np.all((a >= 0) & (a < N) & (p >= 0) & (p < N) & (n >= 0) & (n < N))
        and len(np.unique(a * N + n)) == T
        and np.all(targets[a] == targets[p])
        and np.all(targets[a] != targets[n])
    )
    if not ok:
        raise NotImplementedError("inputs violate mined-triplet structure")

    nc = _get_program()
    in_maps = _shard_inputs(samples, a, p, n)
    res = run_bass_kernel_spmd(nc, in_maps, list(range(NCORES)), trace=_want_trace)
    total = sum(
        float(res.results[c]["out"].astype(np.float64).sum()) for c in range(NCORES)
    )
    loss = np.float32(total / T)
    if _want_trace:
        return loss, res
    return loss
